# revision 16
# baseline (speedup 1.0000x reference)
"""Trainium2 Bass kernel for nn_AttnLayer_60636348285537.

Computes o = einsum('nt,bcthw->bcn', f, video) / (W*H) with the gaussian
attention filters f derived from mu_t/sigma_t, returning [B, C*N].

Sharding: pure data parallel over batch — B=8 batches on 8 NeuronCores,
one batch per core. Each core reduces its [C=1024, T*W*H=6272] slab:
  stage 1 (DVE): vs[c, t]  = sum_wh video[c, t, wh]      (free-dim reduce)
  stage 2 (DVE): out[c, n] = sum_t  vs[c, t] * fs[n, t]  (fs = f/196)
The tiny filter tensor fs is computed on host and replicated to all cores.
"""

import sys

sys.path.insert(0, "/opt/trn_rl_repo")

import numpy as np

P = 128          # SBUF partitions
C = 1024         # channels
T = 32           # time
WH = 196         # W*H = 14*14
X = T * WH       # free elems per channel
N = 3            # gaussian filters
N_CT = C // P    # channel tiles per core
N_CORES = 8

_cache = {}


def _build_module(vid_bufs=4, dma="gpsimd", splits=1, repeats=1,
                  incr_stage2=False, alt_engines=False, s2_chunk=None):
    """splits: sub-DMAs per 128-channel tile (1, 2 or 4; must divide T)."""
    import concourse.bacc as bacc
    import concourse.mybir as mybir
    from concourse import tile

    f32 = mybir.dt.float32
    nc = bacc.Bacc("TRN2", target_bir_lowering=False, debug=False,
                   num_devices=N_CORES)
    vid = nc.dram_tensor("video", [C, X], f32, kind="ExternalInput").ap()
    fw = nc.dram_tensor("fw", [P, N * T], f32, kind="ExternalInput").ap()
    out = nc.dram_tensor("out", [C, N], f32, kind="ExternalOutput").ap()

    dma_eng = {"gpsimd": nc.gpsimd, "sync": nc.sync, "scalar": nc.scalar}[dma]
    engines = ([nc.sync, nc.scalar] if alt_engines else [dma_eng])
    assert T % splits == 0
    ts = T // splits          # t's per DMA group
    xs = X // splits          # free elems per DMA group

    with tile.TileContext(nc) as tc:
        with (
            tc.tile_pool(name="vid", bufs=vid_bufs) as vid_pool,
            tc.tile_pool(name="persist", bufs=1) as persist,
            tc.tile_pool(name="tmp", bufs=2) as tmp_pool,
        ):
            f_sb = persist.tile([P, N * T], f32, tag="f_sb")
            f_view = f_sb.rearrange("p (n t) -> p n t", n=N)

            vid_view = vid.rearrange("(ct p) (s x) -> ct s p x",
                                     p=P, s=splits)
            first = True
            gi = 0
            for _rep in range(repeats):
                vs_all = persist.tile([P, N_CT * T], f32, tag="vs_all")
                out_sb = persist.tile([P, N_CT * N], f32, tag="out_sb")
                vs_view = vs_all.rearrange("p (ct t) -> p ct t", t=T)
                out_view = out_sb.rearrange("p (ct n) -> p ct n", n=N)

                def stage2(ct_list):
                    # out[c, n] = sum_t vs[c, ct, t] * fs[n, t]
                    nct = len(ct_list)
                    ct0 = ct_list[0]
                    prod = tmp_pool.tile([P, nct * T], f32, tag="prod")
                    prod_view = prod.rearrange("p (ct t) -> p ct t", t=T)
                    for n in range(N):
                        f_b = f_view[:, n, :].unsqueeze(1).broadcast_to(
                            [P, nct, T])
                        nc.vector.tensor_mul(
                            prod_view[:], vs_view[:, ct0:ct0 + nct, :], f_b)
                        nc.vector.reduce_sum(
                            out_view[:, ct0:ct0 + nct, n], prod_view[:],
                            axis=mybir.AxisListType.X,
                        )

                for ct in range(N_CT):
                    for s in range(splits):
                        vt = vid_pool.tile([P, xs], f32, tag="vt")
                        engines[gi % len(engines)].dma_start(
                            vt[:], vid_view[ct, s])
                        gi += 1
                        if first:
                            # load the tiny filter tile after the first
                            # video DMA is in flight
                            dma_eng.dma_start(f_sb[:], fw[:])
                            first = False
                        o = ct * T + s * ts
                        nc.vector.reduce_sum(
                            vs_all[:, o:o + ts],
                            vt.rearrange("p (q w) -> p q w", w=WH),
                            axis=mybir.AxisListType.X,
                        )
                    if incr_stage2:
                        stage2([ct])
                    elif s2_chunk and (ct + 1) % s2_chunk == 0:
                        stage2(list(range(ct + 1 - s2_chunk, ct + 1)))
                if not incr_stage2 and not s2_chunk:
                    stage2(list(range(N_CT)))

                dma_eng.dma_start(
                    out.rearrange("(ct p) n -> p ct n", p=P), out_view[:]
                )
    nc.compile()
    return nc


BEST = dict(vid_bufs=12, dma="sync", splits=4, s2_chunk=2)


def _get_module():
    if "nc" not in _cache:
        _cache["nc"] = _build_module(**BEST)
    return _cache["nc"]


def _filters_scaled(mu_t: np.ndarray, sigma_t: np.ndarray) -> np.ndarray:
    """f / (W*H) as [N, T] float32, matching the reference filter math."""
    mu = np.tanh(mu_t.astype(np.float64))
    sg = 1.0 / (1.0 + np.exp(-sigma_t.astype(np.float64)))
    sigma = np.exp(1.5 - 2.0 * sg)
    centers = (T - 1) * (mu + 1.0) / 2.0
    t = np.arange(T, dtype=np.float64)[None, :] - centers[:, None]
    f = np.exp(-(t**2) / (2.0 * sigma[:, None] ** 2 + 1e-16))
    f = f / (np.sum(f, axis=1, keepdims=True) + 1e-16)
    return (f / WH).astype(np.float32)


def kernel(video: np.ndarray, mu_t: np.ndarray, sigma_t: np.ndarray,
           meta: np.ndarray) -> np.ndarray:
    from concourse import bass_utils

    B = video.shape[0]
    assert B == N_CORES, f"kernel hardcodes one batch per core, got B={B}"
    fs = _filters_scaled(np.asarray(mu_t), np.asarray(sigma_t))
    fw = np.tile(fs.reshape(1, N * T), (P, 1))
    vid = np.ascontiguousarray(np.asarray(video), dtype=np.float32)
    vid = vid.reshape(B, C, X)

    nc = _get_module()
    in_maps = [{"video": vid[b], "fw": fw} for b in range(B)]
    res = bass_utils.run_bass_kernel_spmd(nc, in_maps,
                                          core_ids=list(range(N_CORES)))
    out = np.stack([res.results[b]["out"].reshape(C * N) for b in range(B)])
    return out.astype(np.float32)
